# revision 11
# baseline (speedup 1.0000x reference)
"""AFT full attention (nn_AFTFullAttention) — 8-core TRN2 Bass kernel.

The reference reshapes the contiguous [B, T, H*HD] qkv projections straight
to [B, H, T, HD] (torch .view), so "head" h is a block of T/H = 256 original
time rows per batch, reinterpreted as a [2048, 128] matrix.  Sharding one
head per core therefore gives each core complete channel rows: the batch
reduction (denom / weighted) is head-local AND the output projection is
row-parallel — no collectives at all.

Per core (head h): own rows r_loc = b*256 + t_loc (4 batches x 256 rows).
  P*T = W*.T.T @ x_own.T     -> [c, row] tiles; exp/sigmoid/bias fused into
                                the PSUM evacuation.
  The AFT view [tau, delta] of a [row, c] matrix has partition(delta) =
  c % 128, so Ṽ.T / sigmoid(Q̃).T / Y.T are pure strided AP views of the
  [c, row] stores; only the exp(K̃) lhsT tiles need PE transposes (64x).
  numerT_b = ek_b.T.T @ ewT  (lhsT = ek tiles [s,128], rhs = exp(wbias.T))
  outT     = woT.T @ Y.T     row-parallel, + bo fused.
All matmuls in float32r (FP22, full PE rate at N=512).
"""

import os
import sys

sys.path.insert(0, "/opt/trn_rl_repo")

import numpy as np

B, T, DIM, H, HD = 4, 2048, 1024, 8, 128
NCORES = 8
TB = T // H          # 256 original rows per (batch, head-block)
RS = B * TB          # 1024 rows owned per core

KT = DIM // 128      # 8 contraction tiles (dim / c)
ST = T // 128        # 16 s-tiles of the AFT contraction
TC2 = T // 512       # 4 tau-chunks of 512
RC = RS // 512       # 2 row-chunks of 512

TRACE = False        # set by test.py for profiling runs


def _install_ntff_hook():
    """The agent image's antenv lacks axon_hooks; recreate it so
    run_bass_kernel_spmd(trace=True) can capture NTFF profiles."""
    import types

    try:
        from antenv.axon_hooks import get_axon_ntff_profile_hook  # noqa: F401
        return
    except ImportError:
        pass
    import antenv

    mod = types.ModuleType("antenv.axon_hooks")
    _h = [None]
    mod.set_axon_ntff_profile_hook = lambda h: _h.__setitem__(0, h)
    mod.get_axon_ntff_profile_hook = lambda: _h[0]
    sys.modules["antenv.axon_hooks"] = mod
    antenv.axon_hooks = mod
    from trn_agent_boot.trn_boot import _ntff_profile_via_ctypes

    mod.set_axon_ntff_profile_hook(
        _ntff_profile_via_ctypes("/opt/axon/libaxon_pjrt.so")
    )


def _build():
    import concourse.bacc as bacc
    import concourse.tile as tile
    import concourse.mybir as mybir

    f32 = mybir.dt.float32
    f32r = mybir.dt.float32r
    AF = mybir.ActivationFunctionType
    ALU = mybir.AluOpType

    nc = bacc.Bacc("TRN2", debug=False, num_devices=NCORES)

    xT = nc.dram_tensor("xT", [DIM, RS], f32, kind="ExternalInput")
    wqT = nc.dram_tensor("wqT", [DIM, DIM], f32, kind="ExternalInput")
    wkT = nc.dram_tensor("wkT", [DIM, DIM], f32, kind="ExternalInput")
    wvT = nc.dram_tensor("wvT", [DIM, DIM], f32, kind="ExternalInput")
    bq = nc.dram_tensor("bq", [DIM, 1], f32, kind="ExternalInput")
    bk = nc.dram_tensor("bk", [DIM, 1], f32, kind="ExternalInput")
    bv = nc.dram_tensor("bv", [DIM, 1], f32, kind="ExternalInput")
    wbT = nc.dram_tensor("wbT", [T, T], f32, kind="ExternalInput")
    woT = nc.dram_tensor("woT", [DIM, DIM], f32, kind="ExternalInput")
    bo = nc.dram_tensor("bo", [DIM, 1], f32, kind="ExternalInput")
    ident = nc.dram_tensor("ident", [128, 128], f32, kind="ExternalInput")
    out = nc.dram_tensor("out", [DIM, RS], f32, kind="ExternalOutput")

    # [c, row] store free-layout: block j (=c//128) at free j*RS + row.
    # AFT view of rows [r0, r0+n): [128(delta), n, 8] with tau = t*8 + j.
    def aft_view(store, r0, n):
        return store.rearrange("p (j r) -> p j r", j=KT)[
            :, :, r0 : r0 + n
        ].transpose([0, 2, 1])

    with tile.TileContext(nc) as tc:
        with (
            tc.tile_pool(name="const", bufs=1) as constp,
            tc.tile_pool(name="pers", bufs=1) as pers,
        ):
            id_sb = constp.tile([128, 128], f32, tag="id")
            nc.sync.dma_start(out=id_sb, in_=ident[:])
            bias_sb = {}
            for nm, tsr in [("bq", bq), ("bk", bk), ("bv", bv), ("bo", bo)]:
                t_ = constp.tile([128, KT], f32, tag=nm, name=f"b_{nm}")
                nc.sync.dma_start(
                    out=t_, in_=tsr[:].rearrange("(a p) o -> p (a o)", p=128)
                )
                bias_sb[nm] = t_

            # own x rows, transposed: [dim, row], 8 partition-tile blocks
            xts = pers.tile([128, KT * RS], f32r, tag="xts")
            for kt in range(KT):
                nc.sync.dma_start(
                    out=xts[:, kt * RS : (kt + 1) * RS],
                    in_=xT[kt * 128 : (kt + 1) * 128, :].bitcast(f32r),
                )

            # [c, row] projection stores
            sq_sb = pers.tile([128, KT * RS], f32r, tag="sq")
            v_sb = pers.tile([128, KT * RS], f32, tag="v")
            eks_sb = pers.tile([128, B * T], f32r, tag="eks")  # [s,delta] blks
            wsum = pers.tile([128, T], f32, tag="wsum")  # becomes wgt in place
            den = pers.tile([128, T], f32, tag="den")

            # ---------------- stage 1: qkv projections -----------------
            with (
                tc.tile_pool(name="s1", bufs=1) as s1p,
                tc.tile_pool(name="s1ps", bufs=1, space="PSUM") as ps1,
            ):
                ek_sb = s1p.tile([128, KT * RS], f32, tag="ekp", bufs=1)
                specs = [
                    ("q", wqT, AF.Sigmoid, "bq", sq_sb),
                    ("k", wkT, AF.Exp, "bk", ek_sb),
                    ("v", wvT, AF.Identity, "bv", v_sb),
                ]
                for j in range(KT):
                    for nm, wt, func, bnm, store in specs:
                        wtile = s1p.tile([128, KT * 128], f32r, tag="wt",
                                         bufs=4, name=f"wt_{nm}_{j}")
                        nc.sync.dma_start(
                            out=wtile.rearrange("p (a d) -> p a d", a=KT),
                            in_=wt[:, j * 128 : (j + 1) * 128]
                            .rearrange("(a p) d -> p a d", p=128)
                            .bitcast(f32r),
                        )
                        for rc in range(RC):
                            psum = ps1.tile([128, 512], f32, tag="qkv", bufs=4,
                                            name=f"ps_{nm}_{j}_{rc}")
                            for kt in range(KT):
                                nc.tensor.matmul(
                                    psum,
                                    wtile[:, kt * 128 : (kt + 1) * 128],
                                    xts[:, kt * RS + rc * 512 :
                                        kt * RS + (rc + 1) * 512],
                                    start=(kt == 0),
                                    stop=(kt == KT - 1),
                                )
                            nc.scalar.activation(
                                out=store[:, j * RS + rc * 512 :
                                          j * RS + (rc + 1) * 512],
                                in_=psum, func=func,
                                bias=bias_sb[bnm][:, j : j + 1],
                            )

                # ek lhsT tiles: PE-transpose the [delta, tau] views
                for b in range(B):
                    for st in range(ST):
                        view = aft_view(ek_sb, b * TB + st * 16, 16)
                        dvt = s1p.tile([128, 128], f32, tag="dvt", bufs=3,
                                       name=f"dvt_{b}_{st}")
                        nc.vector.tensor_copy(
                            out=dvt.rearrange("p (a c) -> p a c", c=8),
                            in_=view,
                        )
                        tp = ps1.tile([128, 128], f32, tag="tr", bufs=2,
                                      name=f"tp_{b}_{st}")
                        nc.tensor.transpose(tp, dvt, id_sb)
                        blk = b * ST + st
                        nc.vector.tensor_copy(
                            out=eks_sb[:, blk * 128 : (blk + 1) * 128],
                            in_=tp,
                        )

            # ---------------- stage 2: AFT numerator + batch reduce ----
            with (
                tc.tile_pool(name="s2", bufs=1) as s2p,
                tc.tile_pool(name="s2ps", bufs=1, space="PSUM") as ps2,
            ):
                for tc2 in range(TC2):
                    tsl = slice(tc2 * 512, (tc2 + 1) * 512)
                    nps = [ps2.tile([128, 512], f32, tag="np", bufs=5,
                                    name=f"np_{tc2}_{b}") for b in range(B)]
                    for st in range(ST):
                        raw = s2p.tile([128, 512], f32, tag="raw", bufs=4)
                        nc.sync.dma_start(
                            out=raw, in_=wbT[st * 128 : (st + 1) * 128, tsl]
                        )
                        ewt = s2p.tile([128, 512], f32r, tag="ew", bufs=4)
                        nc.scalar.activation(out=ewt, in_=raw, func=AF.Exp)
                        for b in range(B):
                            blk = b * ST + st
                            nc.tensor.matmul(
                                nps[b],
                                eks_sb[:, blk * 128 : (blk + 1) * 128],
                                ewt,
                                start=(st == 0),
                                stop=(st == ST - 1),
                            )
                    for b in range(B):
                        vview = aft_view(v_sb, b * TB + tc2 * 64, 64)
                        npv = nps[b].rearrange("p (a c) -> p a c", c=8)
                        wsv = wsum[:, tsl].rearrange("p (a c) -> p a c", c=8)
                        if b == 0:
                            nc.vector.tensor_tensor(
                                out=wsv, in0=npv, in1=vview, op=ALU.mult,
                            )
                            nc.vector.tensor_copy(out=den[:, tsl], in_=nps[b])
                        else:
                            nv = s2p.tile([128, 512], f32, tag="nv", bufs=3)
                            nc.vector.tensor_tensor(
                                out=nv.rearrange("p (a c) -> p a c", c=8),
                                in0=npv, in1=vview, op=ALU.mult,
                            )
                            nc.vector.tensor_add(
                                out=wsum[:, tsl], in0=wsum[:, tsl], in1=nv
                            )
                            nc.vector.tensor_add(
                                out=den[:, tsl], in0=den[:, tsl], in1=nps[b]
                            )

                rec = s2p.tile([128, T], f32, tag="rec", bufs=1)
                nc.vector.reciprocal(out=rec, in_=den)
                nc.vector.tensor_tensor(out=wsum, in0=wsum, in1=rec,
                                        op=ALU.mult)  # wsum -> weighted

                # y = sigmoid(q̃) * weighted, in place over the sq store
                for b in range(B):
                    for tc2 in range(TC2):
                        tsl = slice(tc2 * 512, (tc2 + 1) * 512)
                        sqv = aft_view(sq_sb, b * TB + tc2 * 64, 64)
                        wgv = wsum[:, tsl].rearrange("p (a c) -> p a c", c=8)
                        nc.vector.tensor_tensor(
                            out=sqv, in0=sqv, in1=wgv, op=ALU.mult,
                        )

            # ---------------- stage 3: out projection ------------------
            with (
                tc.tile_pool(name="s3", bufs=1) as s3p,
                tc.tile_pool(name="s3ps", bufs=1, space="PSUM") as ps3,
            ):
                wo_sb = s3p.tile([128, KT * DIM], f32r, tag="wo", bufs=1)
                nc.sync.dma_start(
                    out=wo_sb.rearrange("p (a d) -> p a d", a=KT),
                    in_=woT[:].rearrange("(a p) d -> p a d", p=128).bitcast(f32r),
                )
                for rc in range(RC):
                    rsl = slice(rc * 512, (rc + 1) * 512)
                    for dt_ in range(KT):
                        pso = ps3.tile([128, 512], f32, tag="o", bufs=3,
                                       name=f"pso_{rc}_{dt_}")
                        for j in range(KT):
                            nc.tensor.matmul(
                                pso,
                                wo_sb[:, (j * KT + dt_) * 128 :
                                      (j * KT + dt_ + 1) * 128],
                                sq_sb[:, j * RS + rc * 512 :
                                      j * RS + (rc + 1) * 512],
                                start=(j == 0),
                                stop=(j == KT - 1),
                            )
                        osb = s3p.tile([128, 512], f32, tag="ot", bufs=3,
                                       name=f"osb_{rc}_{dt_}")
                        nc.scalar.activation(
                            out=osb, in_=pso, func=AF.Identity,
                            bias=bias_sb["bo"][:, dt_ : dt_ + 1],
                        )
                        nc.sync.dma_start(
                            out=out[dt_ * 128 : (dt_ + 1) * 128, rsl], in_=osb
                        )

    nc.compile()
    return nc


_NC_CACHE = None


def kernel(x, Wq, bq, Wk, bk, Wv, bv, wbias, Wo, bo):
    global _NC_CACHE
    from concourse import bass_utils

    f = np.float32
    x = np.asarray(x, f)
    Wq, Wk, Wv, Wo = (np.asarray(a, f) for a in (Wq, Wk, Wv, Wo))
    bq, bk, bv, bo = (np.asarray(a, f) for a in (bq, bk, bv, bo))
    wbias = np.asarray(wbias, f)

    x2 = x.reshape(B * T, DIM)
    wqT = np.ascontiguousarray(Wq.T)
    wkT = np.ascontiguousarray(Wk.T)
    wvT = np.ascontiguousarray(Wv.T)
    woT = np.ascontiguousarray(Wo.T)
    id_np = np.eye(128, dtype=f)
    bqc = np.ascontiguousarray(bq.reshape(DIM, 1))
    bkc = np.ascontiguousarray(bk.reshape(DIM, 1))
    bvc = np.ascontiguousarray(bv.reshape(DIM, 1))
    boc = np.ascontiguousarray(bo.reshape(DIM, 1))

    in_maps = []
    for c in range(NCORES):
        rows = np.concatenate(
            [x2[b * T + c * TB : b * T + (c + 1) * TB] for b in range(B)]
        )  # [RS, DIM], row = b*TB + t_loc
        in_maps.append({
            "xT": np.ascontiguousarray(rows.T),
            "wqT": wqT, "wkT": wkT, "wvT": wvT,
            "bq": bqc, "bk": bkc, "bv": bvc,
            "wbT": np.ascontiguousarray(wbias[c].T),
            "woT": woT, "bo": boc, "ident": id_np,
        })

    if TRACE:
        _install_ntff_hook()
    if _NC_CACHE is None:
        _NC_CACHE = _build()
    nc = _NC_CACHE

    res = bass_utils.run_bass_kernel_spmd(
        nc, in_maps, core_ids=list(range(NCORES)), trace=TRACE
    )
    outf = np.empty((B * T, DIM), f)
    for c in range(NCORES):
        blk = res.results[c]["out"].T  # [RS, DIM], row = b*TB + t_loc
        for b in range(B):
            outf[b * T + c * TB : b * T + (c + 1) * TB] = (
                blk[b * TB : (b + 1) * TB]
            )
    if TRACE:
        kernel.last_exec_time_ns = res.exec_time_ns
        kernel.last_results = res
    return outf.reshape(B, T, DIM)


# revision 13
# speedup vs baseline: 1.0448x; 1.0448x over previous
"""AFT full attention (nn_AFTFullAttention) — 8-core TRN2 Bass kernel.

The reference reshapes the contiguous [B, T, H*HD] qkv projections straight
to [B, H, T, HD] (torch .view), so "head" h is a block of T/H = 256 original
time rows per batch, reinterpreted as a [2048, 128] matrix.  Sharding one
head per core therefore gives each core complete channel rows: the batch
reduction (denom / weighted) is head-local AND the output projection is
row-parallel — no collectives at all.

Per core (head h): own rows r_loc = b*256 + t_loc (4 batches x 256 rows).
  P*T = W*.T.T @ x_own.T     -> [c, row] tiles; exp/sigmoid/bias fused into
                                the PSUM evacuation.
  The AFT view [tau, delta] of a [row, c] matrix has partition(delta) =
  c % 128, so Ṽ.T / sigmoid(Q̃).T / Y.T are pure strided AP views of the
  [c, row] stores; only the exp(K̃) lhsT tiles need PE transposes (64x).
  numerT_b = ek_b.T.T @ ewT  (lhsT = ek tiles [s,128], rhs = exp(wbias.T))
  outT     = woT.T @ Y.T     row-parallel, + bo fused.
All matmuls in float32r (FP22, full PE rate at N=512).
"""

import os
import sys

sys.path.insert(0, "/opt/trn_rl_repo")

import numpy as np

B, T, DIM, H, HD = 4, 2048, 1024, 8, 128
NCORES = 8
TB = T // H          # 256 original rows per (batch, head-block)
RS = B * TB          # 1024 rows owned per core

KT = DIM // 128      # 8 contraction tiles (dim / c)
ST = T // 128        # 16 s-tiles of the AFT contraction
TC2 = T // 512       # 4 tau-chunks of 512
RC = RS // 512       # 2 row-chunks of 512

TRACE = False        # set by test.py for profiling runs


def _install_ntff_hook():
    """The agent image's antenv lacks axon_hooks; recreate it so
    run_bass_kernel_spmd(trace=True) can capture NTFF profiles."""
    import types

    try:
        from antenv.axon_hooks import get_axon_ntff_profile_hook  # noqa: F401
        return
    except ImportError:
        pass
    import antenv

    mod = types.ModuleType("antenv.axon_hooks")
    _h = [None]
    mod.set_axon_ntff_profile_hook = lambda h: _h.__setitem__(0, h)
    mod.get_axon_ntff_profile_hook = lambda: _h[0]
    sys.modules["antenv.axon_hooks"] = mod
    antenv.axon_hooks = mod
    from trn_agent_boot.trn_boot import _ntff_profile_via_ctypes

    mod.set_axon_ntff_profile_hook(
        _ntff_profile_via_ctypes("/opt/axon/libaxon_pjrt.so")
    )


def _build():
    import concourse.bacc as bacc
    import concourse.tile as tile
    import concourse.mybir as mybir

    f32 = mybir.dt.float32
    f32r = mybir.dt.float32r
    AF = mybir.ActivationFunctionType
    ALU = mybir.AluOpType

    nc = bacc.Bacc("TRN2", debug=False, num_devices=NCORES)

    xT = nc.dram_tensor("xT", [DIM, RS], f32, kind="ExternalInput")
    wqT = nc.dram_tensor("wqT", [128, KT * DIM], f32, kind="ExternalInput")
    wkT = nc.dram_tensor("wkT", [128, KT * DIM], f32, kind="ExternalInput")
    wvT = nc.dram_tensor("wvT", [128, KT * DIM], f32, kind="ExternalInput")
    bq = nc.dram_tensor("bq", [DIM, 1], f32, kind="ExternalInput")
    bk = nc.dram_tensor("bk", [DIM, 1], f32, kind="ExternalInput")
    bv = nc.dram_tensor("bv", [DIM, 1], f32, kind="ExternalInput")
    wbT = nc.dram_tensor("wbT", [T, T], f32, kind="ExternalInput")
    woT = nc.dram_tensor("woT", [128, KT * DIM], f32, kind="ExternalInput")
    bo = nc.dram_tensor("bo", [DIM, 1], f32, kind="ExternalInput")
    ident = nc.dram_tensor("ident", [128, 128], f32, kind="ExternalInput")
    out = nc.dram_tensor("out", [DIM, RS], f32, kind="ExternalOutput")

    # [c, row] store free-layout: block j (=c//128) at free j*RS + row.
    # AFT view of rows [r0, r0+n): [128(delta), n, 8] with tau = t*8 + j.
    def aft_view(store, r0, n):
        return store.rearrange("p (j r) -> p j r", j=KT)[
            :, :, r0 : r0 + n
        ].transpose([0, 2, 1])

    with tile.TileContext(nc) as tc:
        with (
            tc.tile_pool(name="const", bufs=1) as constp,
            tc.tile_pool(name="pers", bufs=1) as pers,
        ):
            id_sb = constp.tile([128, 128], f32, tag="id")
            nc.sync.dma_start(out=id_sb, in_=ident[:])
            bias_sb = {}
            for nm, tsr in [("bq", bq), ("bk", bk), ("bv", bv), ("bo", bo)]:
                t_ = constp.tile([128, KT], f32, tag=nm, name=f"b_{nm}")
                nc.sync.dma_start(
                    out=t_, in_=tsr[:].rearrange("(a p) o -> p (a o)", p=128)
                )
                bias_sb[nm] = t_

            # own x rows, transposed: [dim, row], 8 partition-tile blocks
            xts = pers.tile([128, KT * RS], f32r, tag="xts")
            for kt in range(KT):
                nc.sync.dma_start(
                    out=xts[:, kt * RS : (kt + 1) * RS],
                    in_=xT[kt * 128 : (kt + 1) * 128, :].bitcast(f32r),
                )

            # [c, row] projection stores
            sq_sb = pers.tile([128, KT * RS], f32r, tag="sq")
            v_sb = pers.tile([128, KT * RS], f32, tag="v")
            eks_sb = pers.tile([128, B * T], f32r, tag="eks")  # [s,delta] blks
            wsum = pers.tile([128, T], f32, tag="wsum")  # becomes wgt in place
            den = pers.tile([128, T], f32, tag="den")

            # ---------------- stage 1: qkv projections -----------------
            with (
                tc.tile_pool(name="s1", bufs=1) as s1p,
                tc.tile_pool(name="s1ps", bufs=1, space="PSUM") as ps1,
            ):
                ek_sb = s1p.tile([128, KT * RS], f32, tag="ekp", bufs=1)
                specs = [
                    ("q", wqT, AF.Sigmoid, "bq", sq_sb),
                    ("k", wkT, AF.Exp, "bk", ek_sb),
                    ("v", wvT, AF.Identity, "bv", v_sb),
                ]
                for j in range(KT):
                    for nm, wt, func, bnm, store in specs:
                        wtile = s1p.tile([128, KT * 128], f32r, tag="wt",
                                         bufs=6, name=f"wt_{nm}_{j}")
                        nc.sync.dma_start(
                            out=wtile,
                            in_=wt[:, j * KT * 128 :
                                   (j + 1) * KT * 128].bitcast(f32r),
                        )
                        for rc in range(RC):
                            psum = ps1.tile([128, 512], f32, tag="qkv", bufs=4,
                                            name=f"ps_{nm}_{j}_{rc}")
                            for kt in range(KT):
                                nc.tensor.matmul(
                                    psum,
                                    wtile[:, kt * 128 : (kt + 1) * 128],
                                    xts[:, kt * RS + rc * 512 :
                                        kt * RS + (rc + 1) * 512],
                                    start=(kt == 0),
                                    stop=(kt == KT - 1),
                                )
                            nc.scalar.activation(
                                out=store[:, j * RS + rc * 512 :
                                          j * RS + (rc + 1) * 512],
                                in_=psum, func=func,
                                bias=bias_sb[bnm][:, j : j + 1],
                            )

                # ek lhsT tiles: PE-transpose the [delta, tau] views
                for b in range(B):
                    for st in range(ST):
                        view = aft_view(ek_sb, b * TB + st * 16, 16)
                        dvt = s1p.tile([128, 128], f32, tag="dvt", bufs=3,
                                       name=f"dvt_{b}_{st}")
                        nc.vector.tensor_copy(
                            out=dvt.rearrange("p (a c) -> p a c", c=8),
                            in_=view,
                        )
                        tp = ps1.tile([128, 128], f32, tag="tr", bufs=2,
                                      name=f"tp_{b}_{st}")
                        nc.tensor.transpose(tp, dvt, id_sb)
                        blk = b * ST + st
                        nc.vector.tensor_copy(
                            out=eks_sb[:, blk * 128 : (blk + 1) * 128],
                            in_=tp,
                        )

            # ---------------- stage 2: AFT numerator + batch reduce ----
            with (
                tc.tile_pool(name="s2", bufs=1) as s2p,
                tc.tile_pool(name="s2ps", bufs=1, space="PSUM") as ps2,
            ):
                for tc2 in range(TC2):
                    tsl = slice(tc2 * 512, (tc2 + 1) * 512)
                    nps = [ps2.tile([128, 512], f32, tag="np", bufs=5,
                                    name=f"np_{tc2}_{b}") for b in range(B)]
                    for st in range(ST):
                        raw = s2p.tile([128, 512], f32, tag="raw", bufs=4)
                        nc.sync.dma_start(
                            out=raw, in_=wbT[st * 128 : (st + 1) * 128, tsl]
                        )
                        ewt = s2p.tile([128, 512], f32r, tag="ew", bufs=4)
                        nc.scalar.activation(out=ewt, in_=raw, func=AF.Exp)
                        for b in range(B):
                            blk = b * ST + st
                            nc.tensor.matmul(
                                nps[b],
                                eks_sb[:, blk * 128 : (blk + 1) * 128],
                                ewt,
                                start=(st == 0),
                                stop=(st == ST - 1),
                            )
                    for b in range(B):
                        vview = aft_view(v_sb, b * TB + tc2 * 64, 64)
                        npv = nps[b].rearrange("p (a c) -> p a c", c=8)
                        wsv = wsum[:, tsl].rearrange("p (a c) -> p a c", c=8)
                        if b == 0:
                            nc.vector.tensor_tensor(
                                out=wsv, in0=npv, in1=vview, op=ALU.mult,
                            )
                            nc.vector.tensor_copy(out=den[:, tsl], in_=nps[b])
                        else:
                            nv = s2p.tile([128, 512], f32, tag="nv", bufs=3)
                            nc.vector.tensor_tensor(
                                out=nv.rearrange("p (a c) -> p a c", c=8),
                                in0=npv, in1=vview, op=ALU.mult,
                            )
                            nc.vector.tensor_add(
                                out=wsum[:, tsl], in0=wsum[:, tsl], in1=nv
                            )
                            nc.vector.tensor_add(
                                out=den[:, tsl], in0=den[:, tsl], in1=nps[b]
                            )

                rec = s2p.tile([128, T], f32, tag="rec", bufs=1)
                nc.vector.reciprocal_approx_fast(out=rec, in_=den)
                nc.vector.tensor_tensor(out=wsum, in0=wsum, in1=rec,
                                        op=ALU.mult)  # wsum -> weighted

                # y = sigmoid(q̃) * weighted, in place over the sq store
                for b in range(B):
                    for tc2 in range(TC2):
                        tsl = slice(tc2 * 512, (tc2 + 1) * 512)
                        sqv = aft_view(sq_sb, b * TB + tc2 * 64, 64)
                        wgv = wsum[:, tsl].rearrange("p (a c) -> p a c", c=8)
                        nc.vector.tensor_tensor(
                            out=sqv, in0=sqv, in1=wgv, op=ALU.mult,
                        )

            # ---------------- stage 3: out projection ------------------
            with (
                tc.tile_pool(name="s3", bufs=1) as s3p,
                tc.tile_pool(name="s3ps", bufs=1, space="PSUM") as ps3,
            ):
                wo_sb = s3p.tile([128, KT * DIM], f32r, tag="wo", bufs=1)
                nc.sync.dma_start(out=wo_sb, in_=woT[:].bitcast(f32r))
                for rc in range(RC):
                    rsl = slice(rc * 512, (rc + 1) * 512)
                    for dt_ in range(KT):
                        pso = ps3.tile([128, 512], f32, tag="o", bufs=3,
                                       name=f"pso_{rc}_{dt_}")
                        for j in range(KT):
                            nc.tensor.matmul(
                                pso,
                                wo_sb[:, (j * KT + dt_) * 128 :
                                      (j * KT + dt_ + 1) * 128],
                                sq_sb[:, j * RS + rc * 512 :
                                      j * RS + (rc + 1) * 512],
                                start=(j == 0),
                                stop=(j == KT - 1),
                            )
                        osb = s3p.tile([128, 512], f32, tag="ot", bufs=3,
                                       name=f"osb_{rc}_{dt_}")
                        nc.scalar.activation(
                            out=osb, in_=pso, func=AF.Identity,
                            bias=bias_sb["bo"][:, dt_ : dt_ + 1],
                        )
                        nc.sync.dma_start(
                            out=out[dt_ * 128 : (dt_ + 1) * 128, rsl], in_=osb
                        )

    nc.compile()
    return nc


_NC_CACHE = None


def kernel(x, Wq, bq, Wk, bk, Wv, bv, wbias, Wo, bo):
    global _NC_CACHE
    from concourse import bass_utils

    f = np.float32
    x = np.asarray(x, f)
    Wq, Wk, Wv, Wo = (np.asarray(a, f) for a in (Wq, Wk, Wv, Wo))
    bq, bk, bv, bo = (np.asarray(a, f) for a in (bq, bk, bv, bo))
    wbias = np.asarray(wbias, f)

    x2 = x.reshape(B * T, DIM)

    def tile_w(W):
        # host[p, j*1024 + kt*128 + d] = W[j*128+d, kt*128+p]
        return np.ascontiguousarray(
            W.reshape(KT, 128, KT, 128).transpose(3, 0, 2, 1).reshape(
                128, KT * KT * 128)
        )

    wqT = tile_w(Wq)
    wkT = tile_w(Wk)
    wvT = tile_w(Wv)
    # wo store is contraction-major: host[p, j*1024 + dt*128 + d]
    #   = Wo.T[j*128+p, dt*128+d] = Wo[dt*128+d, j*128+p]
    woT = np.ascontiguousarray(
        Wo.reshape(KT, 128, KT, 128).transpose(3, 2, 0, 1).reshape(
            128, KT * KT * 128)
    )
    id_np = np.eye(128, dtype=f)
    bqc = np.ascontiguousarray(bq.reshape(DIM, 1))
    bkc = np.ascontiguousarray(bk.reshape(DIM, 1))
    bvc = np.ascontiguousarray(bv.reshape(DIM, 1))
    boc = np.ascontiguousarray(bo.reshape(DIM, 1))

    in_maps = []
    for c in range(NCORES):
        rows = np.concatenate(
            [x2[b * T + c * TB : b * T + (c + 1) * TB] for b in range(B)]
        )  # [RS, DIM], row = b*TB + t_loc
        in_maps.append({
            "xT": np.ascontiguousarray(rows.T),
            "wqT": wqT, "wkT": wkT, "wvT": wvT,
            "bq": bqc, "bk": bkc, "bv": bvc,
            "wbT": np.ascontiguousarray(wbias[c].T),
            "woT": woT, "bo": boc, "ident": id_np,
        })

    if TRACE:
        _install_ntff_hook()
    if _NC_CACHE is None:
        _NC_CACHE = _build()
    nc = _NC_CACHE

    res = bass_utils.run_bass_kernel_spmd(
        nc, in_maps, core_ids=list(range(NCORES)), trace=TRACE
    )
    outf = np.empty((B * T, DIM), f)
    for c in range(NCORES):
        blk = res.results[c]["out"].T  # [RS, DIM], row = b*TB + t_loc
        for b in range(B):
            outf[b * T + c * TB : b * T + (c + 1) * TB] = (
                blk[b * TB : (b + 1) * TB]
            )
    if TRACE:
        kernel.last_exec_time_ns = res.exec_time_ns
        kernel.last_results = res
    return outf.reshape(B, T, DIM)
